# revision 7
# baseline (speedup 1.0000x reference)
"""CAM_Module (channel attention) Trainium2 Bass kernel.

x: (16, 512, 64, 64) f32, gamma: (1,) f32
  xf = x.reshape(B, C, N)           N = 4096
  energy = xf @ xf^T                (B, C, C)
  att = softmax(max(energy) - energy, axis=-1)   == softmax(-energy) (shift-invariant)
  out = gamma * (att @ xf) + x

Sharding: data-parallel over batch, 2 batches per core on 8 cores.

Per-core pipeline (per batch):
  - SWDGE cast-load x -> SBUF as f32r (rounded fp32, full-rate matmul dtype)
  - PE transpose-mode: xf^T chunks (n on partitions) -> PSUM -> ACT copy -> SBUF
  - MM1 (f32r): energy[i, :] accumulated over 32 k-chunks into 4 PSUM banks
  - softmax: DVE row-min, ACT exp(min - e) with row-sum accumulation,
    DVE reciprocal, scale by gamma/Z
  - PE transpose att -> att^T
  - MM2 (f32r): att^T.T @ xf accumulated over 4 j-chunks, DVE adds residual x
  - HWDGE store
"""

import sys

if "/opt/trn_rl_repo" not in sys.path:
    sys.path.insert(0, "/opt/trn_rl_repo")

from contextlib import ExitStack

import numpy as np

import concourse.bass as bass
import concourse.tile as tile
from concourse import bacc, mybir
from concourse.bass_utils import run_bass_kernel_spmd
from concourse.masks import make_identity

N_CORES = 8
B, C, H, W = 16, 512, 64, 64
N = H * W                    # 4096
BPC = B // N_CORES           # batches per core = 2
CT = C // 128                # 4 c-tiles
KT = N // 128                # 32 k-chunks (transposed layout)
NCH = N // 512               # 8 moving chunks for MM2

F32 = mybir.dt.float32
F32R = mybir.dt.float32r


def _build_nc():
    nc = bacc.Bacc("TRN2", target_bir_lowering=False, debug=False,
                   num_devices=N_CORES)
    x_d = nc.dram_tensor("x", [BPC, C, N], F32, kind="ExternalInput").ap()
    g_d = nc.dram_tensor("gamma", [1], F32, kind="ExternalInput").ap()
    o_d = nc.dram_tensor("out", [BPC, C, N], F32, kind="ExternalOutput").ap()

    with tile.TileContext(nc) as tc, ExitStack() as ctx:
        xf_pool = ctx.enter_context(tc.tile_pool(name="xf", bufs=BPC * CT))
        xfT_pool = ctx.enter_context(tc.tile_pool(name="xfT", bufs=6))
        s_pool = ctx.enter_context(tc.tile_pool(name="s", bufs=CT))
        att_pool = ctx.enter_context(tc.tile_pool(name="att", bufs=CT))
        attT_pool = ctx.enter_context(tc.tile_pool(name="attT", bufs=CT))
        out_pool = ctx.enter_context(tc.tile_pool(name="outp", bufs=2))
        stat_pool = ctx.enter_context(tc.tile_pool(name="stat", bufs=4 * CT))
        one_pool = ctx.enter_context(tc.tile_pool(name="one", bufs=1))
        pT = ctx.enter_context(tc.tile_pool(name="pT", bufs=2, space="PSUM"))
        pE = ctx.enter_context(tc.tile_pool(name="pE", bufs=CT, space="PSUM"))
        pO = ctx.enter_context(tc.tile_pool(name="pO", bufs=2, space="PSUM"))

        # identity for PE transpose-mode (f32r so dtypes match the data)
        ident_f = one_pool.tile([128, 128], F32, tag="idf")
        make_identity(nc, ident_f[:])
        ident = one_pool.tile([128, 128], F32R, tag="idr")
        nc.vector.tensor_copy(ident[:], ident_f[:])

        # broadcast gamma to all 128 partitions via K=1 matmul with ones
        g_sb = one_pool.tile([1, 1], F32, tag="gsb")
        nc.sync.dma_start(g_sb[:], g_d.rearrange("(a b) -> a b", a=1))
        ones = one_pool.tile([1, 128], F32, tag="ones")
        nc.vector.memset(ones[:], 1.0)
        pG = pT.tile([128, 1], F32, tag="pt", name="pG")
        nc.tensor.matmul(pG[:], ones[:], g_sb[:], start=True, stop=True)
        g_bc = one_pool.tile([128, 1], F32, tag="gbc")
        nc.vector.tensor_copy(g_bc[:], pG[:])

        for b in range(BPC):
            # ---- load x (cast f32 -> f32r during DMA) ----
            xf = []
            for ct in range(CT):
                t = xf_pool.tile([128, N], F32R, tag="xf")
                nc.gpsimd.dma_start(t[:], x_d[b, ct * 128:(ct + 1) * 128, :])
                xf.append(t)

            # ---- transpose to (n, c) layout + Gram matmuls ----
            e_ps = [
                pE.tile([128, C], F32, tag="pe", name=f"pe_{b}_{i}")
                for i in range(CT)
            ]
            for k in range(KT):
                tp = pT.tile([128, C], F32R, tag="pt")
                for ct in range(CT):
                    nc.tensor.transpose(
                        tp[:, ct * 128:(ct + 1) * 128],
                        xf[ct][:, k * 128:(k + 1) * 128],
                        ident[:],
                    )
                xT = xfT_pool.tile([128, C], F32R, tag="xT")
                nc.scalar.copy(xT[:], tp[:])
                for it in range(CT):
                    nc.tensor.matmul(
                        e_ps[it][:],
                        xT[:, it * 128:(it + 1) * 128],
                        xT[:],
                        start=(k == 0),
                        stop=(k == KT - 1),
                    )

            # ---- softmax(-energy) rows, scaled by gamma ----
            att = []
            for it in range(CT):
                m = stat_pool.tile([128, 1], F32, tag="m")
                nc.vector.tensor_reduce(
                    m[:], e_ps[it][:], axis=mybir.AxisListType.X,
                    op=mybir.AluOpType.min,
                )
                s = s_pool.tile([128, C], F32, tag="s")
                z = stat_pool.tile([128, 1], F32, tag="z")
                nc.scalar.activation(
                    s[:], e_ps[it][:], mybir.ActivationFunctionType.Exp,
                    bias=m[:], scale=-1.0, accum_out=z[:],
                )
                rz = stat_pool.tile([128, 1], F32, tag="rz")
                nc.vector.reciprocal(rz[:], z[:])
                g = stat_pool.tile([128, 1], F32, tag="g")
                nc.vector.tensor_mul(g[:], rz[:], g_bc[:])
                a = att_pool.tile([128, C], F32R, tag="a")
                nc.vector.tensor_scalar_mul(a[:], s[:], g[:])
                att.append(a)

            # ---- transpose attention ----
            attT = []
            for jt in range(CT):
                tp = pT.tile([128, C], F32R, tag="pt")
                for it in range(CT):
                    nc.tensor.transpose(
                        tp[:, it * 128:(it + 1) * 128],
                        att[it][:, jt * 128:(jt + 1) * 128],
                        ident[:],
                    )
                aT = attT_pool.tile([128, C], F32R, tag="aT")
                nc.vector.tensor_copy(aT[:], tp[:])
                attT.append(aT)

            # ---- out = att @ xf + x ----
            for it in range(CT):
                o = out_pool.tile([128, N], F32, tag="o")
                for nch in range(NCH):
                    po = pO.tile([128, 512], F32, tag="po")
                    for jt in range(CT):
                        nc.tensor.matmul(
                            po[:],
                            attT[jt][:, it * 128:(it + 1) * 128],
                            xf[jt][:, nch * 512:(nch + 1) * 512],
                            start=(jt == 0),
                            stop=(jt == CT - 1),
                        )
                    nc.vector.tensor_add(
                        o[:, nch * 512:(nch + 1) * 512], po[:],
                        xf[it][:, nch * 512:(nch + 1) * 512],
                    )
                nc.sync.dma_start(o_d[b, it * 128:(it + 1) * 128, :], o[:])

    nc.compile()
    return nc


_NC = None


def _get_nc():
    global _NC
    if _NC is None:
        _NC = _build_nc()
    return _NC


def kernel(x, gamma):
    assert x.shape == (B, C, H, W) and x.dtype == np.float32
    nc = _get_nc()
    xf = np.ascontiguousarray(np.asarray(x, np.float32).reshape(B, C, N))
    g = np.ascontiguousarray(np.asarray(gamma, np.float32).reshape(1))
    in_maps = [
        {"x": xf[c * BPC:(c + 1) * BPC], "gamma": g} for c in range(N_CORES)
    ]
    res = run_bass_kernel_spmd(nc, in_maps, core_ids=list(range(N_CORES)))
    out = np.concatenate(
        [res.results[c]["out"] for c in range(N_CORES)], axis=0
    )
    return out.reshape(B, C, H, W).astype(np.float32)
